# revision 2
# baseline (speedup 1.0000x reference)
"""Trainium2 Bass kernel for nn_ClassDiagramGNN: 2-layer GAT on 50k nodes / 850k edges.

Strategy (8 NeuronCores, dst-sharded graph parallel):
  - Host: add self-loops, balance dst nodes into 128-wide blocks per core (LPT
    by degree), permute node ids so each block is contiguous, bucket edges by
    (core, block, src-half) and pad to 128-edge tiles. Precompute the one-hot
    edge->dst scatter matrix S and its transpose ST per tile in bf16 (loaded
    by DMA, not built on-chip).
  - Phase A: each core computes its shard of h1 = x @ W1 plus the per-node
    attention scalars (folded into the matmul via precombined weights), packed
    as 640-col bf16 rows (1280B) -> AllGather full table. Per-node adst
    scalars also go to a compact local table.
  - Phase B: per dst block, dma_gather the source rows for the block's edges
    (int16 indices, table split in two 25k-row halves), ae = ST @ adst
    accumulated across tiles in PSUM, then one batched add/leaky/exp over
    [128, T*NH], per-tile w = p * h (split across ACT and DVE engines),
    aggregate via S^T @ w matmuls accumulated in PSUM (bf16, 1 PE pass),
    normalize by the S^T @ p denominator, ELU, then h2 @ W2 -> 256-col bf16
    rows -> AllGather.
  - Phase D: same edge pass for layer 2 -> fp32 output shard; host unpermutes.
"""
import sys

for _p in ("/opt/trn_rl_repo",):
    if _p not in sys.path:
        sys.path.append(_p)

import heapq
import numpy as np
import ml_dtypes

import concourse.bass as bass
import concourse.bacc as bacc
import concourse.tile as tile
from concourse import mybir
from concourse import bass_utils

F32 = mybir.dt.float32
BF16 = mybir.dt.bfloat16
I16 = mybir.dt.int16
AF = mybir.ActivationFunctionType
OP = mybir.AluOpType
NPBF = ml_dtypes.bfloat16

# problem constants (hardcoded per contract)
N, F_IN, HID, H1, E = 50000, 512, 128, 4, 800000
NEG = 0.2
C = 8                 # cores
NS = N // C           # 6250 nodes per shard
NBLK = (NS + 127) // 128   # 49 blocks per core
CAPS = [128] * (NBLK - 1) + [NS - 128 * (NBLK - 1)]  # 48x128 + 106
HALF = N // 2         # gather-table half split (int16 index reach)
ROW1 = 640            # layer-1 bf16 row: 512 feat + 4 asrc + pad (1280B, 256B-aligned)
ROW2 = 256            # layer-2 bf16 row: 128 feat + 1 asrc + pad (512B)
EPS = 1e-16

_cache = {}


def _reconfigure(n, e):
    """Testing hook: shrink the graph (keeps F_IN/HID/H1 fixed)."""
    global N, E, NS, NBLK, CAPS, HALF
    N, E = n, e
    NS = N // C
    NBLK = (NS + 127) // 128
    CAPS = [128] * (NBLK - 1) + [NS - 128 * (NBLK - 1)]
    HALF = N // 2
    _cache.clear()


# --------------------------------------------------------------------------
# host-side preprocessing
# --------------------------------------------------------------------------

def _prepare(x, edge_index, W1, a_src1, a_dst1, b1, W2, a_src2, a_dst2, b2):
    src = np.concatenate([edge_index[0].astype(np.int64), np.arange(N, dtype=np.int64)])
    dst = np.concatenate([edge_index[1].astype(np.int64), np.arange(N, dtype=np.int64)])
    deg = np.bincount(dst, minlength=N)

    # per-core LPT balance of dst nodes into blocks (by degree)
    perm_pos = np.empty(N, dtype=np.int64)    # orig id -> permuted global pos
    perm_order = np.empty(N, dtype=np.int64)  # permuted pos -> orig id
    for c in range(C):
        ids = np.arange(c * NS, (c + 1) * NS)
        d = deg[ids]
        order = np.argsort(-d, kind="stable")
        heap = [(0, 0, i) for i in range(NBLK)]
        heapq.heapify(heap)
        assign = [[] for _ in range(NBLK)]
        for lid in order:
            while True:
                load, used, bi = heapq.heappop(heap)
                if used < CAPS[bi]:
                    break
            assign[bi].append(lid)
            heapq.heappush(heap, (load + int(d[lid]), used + 1, bi))
        pos = 0
        for bi in range(NBLK):
            for lid in assign[bi]:
                g = c * NS + pos
                perm_pos[c * NS + lid] = g
                perm_order[g] = c * NS + lid
                pos += 1

    src_p = perm_pos[src]
    dst_p = perm_pos[dst]
    core = dst_p // NS
    blk = (dst_p % NS) // 128
    halfv = (src_p >= HALF).astype(np.int64)
    key = (core * NBLK + blk) * 2 + halfv
    eorder = np.argsort(key, kind="stable")
    counts = np.bincount(key, minlength=C * NBLK * 2).reshape(C, NBLK, 2)

    # cross-core uniform tile counts per block
    T_lo = -(-counts[:, :, 0].max(axis=0) // 128)  # ceil
    T_hi = -(-counts[:, :, 1].max(axis=0) // 128)
    T_all = T_lo + T_hi
    TT = int(T_all.sum())
    toff = np.zeros(NBLK, np.int64)
    toff[1:] = np.cumsum(T_all)[:-1]

    src_sorted = src_p[eorder]
    dloc_sorted = (dst_p[eorder] % NS) % 128
    starts = np.zeros(C * NBLK * 2 + 1, np.int64)
    starts[1:] = np.cumsum(counts.reshape(-1))

    idx_all = np.zeros((C, TT * 128), np.int16)           # pad -> row 0
    dc_all = np.full((C, TT * 128), 999, np.int64)        # pad -> no dst
    for c in range(C):
        for b in range(NBLK):
            for h in range(2):
                k = (c * NBLK + b) * 2 + h
                s0, s1 = starts[k], starts[k + 1]
                n = s1 - s0
                if n == 0:
                    continue
                slot0 = (toff[b] + (T_lo[b] if h else 0)) * 128
                seg = src_sorted[s0:s1]
                if h:
                    seg = seg - HALF
                idx_all[c, slot0:slot0 + n] = seg.astype(np.int16)
                dc_all[c, slot0:slot0 + n] = dloc_sorted[s0:s1]

    # weights: fold per-head attention projections into the linear transforms
    W1_64 = np.asarray(W1, np.float64)
    Dsrc1 = np.zeros((H1 * HID, H1), np.float64)
    Ddst1 = np.zeros((H1 * HID, H1), np.float64)
    a_src1_64 = np.asarray(a_src1, np.float64)
    a_dst1_64 = np.asarray(a_dst1, np.float64)
    for h in range(H1):
        Dsrc1[h * HID:(h + 1) * HID, h] = a_src1_64[h]
        Ddst1[h * HID:(h + 1) * HID, h] = a_dst1_64[h]
    rhs1 = np.concatenate(
        [np.asarray(W1, np.float32),
         (W1_64 @ Dsrc1).astype(np.float32),
         (W1_64 @ Ddst1).astype(np.float32)], axis=1).astype(NPBF)  # [512, 520]
    W2_64 = np.asarray(W2, np.float64)
    rhs2 = np.concatenate(
        [np.asarray(W2, np.float32),
         (W2_64 @ np.asarray(a_src2, np.float64)[0][:, None]).astype(np.float32),
         (W2_64 @ np.asarray(a_dst2, np.float64)[0][:, None]).astype(np.float32)],
        axis=1).astype(NPBF)                                        # [512, 130]

    ident = np.eye(128, dtype=NPBF)
    b1r = np.tile(np.asarray(b1, np.float32)[None, :], (128, 1))
    b2r = np.tile(np.asarray(b2, np.float32)[None, :], (128, 1))

    # one-hot scatter matrices, one 128x128 tile per edge tile (bf16)
    slots = np.arange(TT * 128)
    tloc = slots // 128
    posi = slots % 128

    xnp = np.asarray(x, np.float32)
    in_maps = []
    for c in range(C):
        rows = perm_order[c * NS:(c + 1) * NS]
        xT = np.ascontiguousarray(xnp[rows].T).astype(NPBF)     # [512, 6250]
        idx_w = np.tile(idx_all[c].reshape(-1, 16).T, (8, 1))   # [128, TT*8]
        dcv = dc_all[c]
        valid = dcv < 128
        dv = dcv[valid]
        tv = tloc[valid]
        pv = posi[valid]
        S_all = np.zeros((128, TT * 128), NPBF)
        S_all[pv, tv * 128 + dv] = 1
        ST_all = np.zeros((128, TT * 128), NPBF)
        ST_all[dv, tv * 128 + pv] = 1
        in_maps.append({
            "xT": xT, "rhs1": rhs1, "rhs2": rhs2,
            "b1r": b1r, "b2r": b2r, "ident": ident,
            "idx": np.ascontiguousarray(idx_w),
            "S": S_all, "ST": ST_all,
        })

    meta = {
        "T_lo": [int(v) for v in T_lo],
        "T_hi": [int(v) for v in T_hi],
        "toff": [int(v) for v in toff],
        "TT": TT,
    }
    return in_maps, meta, perm_order


# --------------------------------------------------------------------------
# device program
# --------------------------------------------------------------------------

def _edge_layer(nc, tc, meta, lay, pools, ad_my, hfull, out_writer):
    """Shared per-block edge pass for both GAT layers.

    lay=1: ROW=640, 4 heads, feat cols 0:512, asrc 512:516
    lay=2: ROW=256, 1 head, feat cols 0:128, asrc 128:129
    out_writer(b, base, bs, oacc, dacc) consumes the block result.
    """
    sbm, sbg, sbs, sbw, psb, psa = pools
    ROW = ROW1 if lay == 1 else ROW2
    NH = H1 if lay == 1 else 1
    FEAT = NH * HID
    ACOL = FEAT            # asrc col start
    idx_d, S_d, ST_d = meta["idx_ap"], meta["S_ap"], meta["ST_ap"]

    for b in range(NBLK):
        bs = CAPS[b]
        base = b * 128
        T_lo, T_hi = meta["T_lo"][b], meta["T_hi"][b]
        T = T_lo + T_hi
        boff = meta["toff"][b]

        idx_sb = sbm.tile([128, T * 8], I16, tag="idx")
        nc.sync.dma_start(idx_sb[:], idx_d[:, boff * 8:(boff + T) * 8])
        S_sb = sbm.tile([128, T * 128], BF16, tag="S")
        nc.sync.dma_start(S_sb[:], S_d[:, boff * 128:(boff + T) * 128])
        ST_sb = sbm.tile([128, T * 128], BF16, tag="ST")
        nc.sync.dma_start(ST_sb[:], ST_d[:, boff * 128:(boff + T) * 128])

        adst_sb = sbm.tile([128, NH], BF16, tag="adst")
        if bs < 128:
            nc.vector.memset(adst_sb[:], 0.0)
        nc.sync.dma_start(adst_sb[:bs], ad_my[base:base + bs, :])

        gat = sbg.tile([128, T, ROW], BF16, tag="gat")
        if T_lo:
            nc.gpsimd.dma_gather(
                gat[:, 0:T_lo, :], hfull[0:HALF, :], idx_sb[:, 0:T_lo * 8],
                T_lo * 128, T_lo * 128, ROW, elem_step=ROW, single_packet=False)
        if T_hi:
            nc.gpsimd.dma_gather(
                gat[:, T_lo:T, :], hfull[HALF:N, :], idx_sb[:, T_lo * 8:T * 8],
                T_hi * 128, T_hi * 128, ROW, elem_step=ROW, single_packet=False)

        # attention scalars, batched over the whole block
        ae = psa.tile([128, T * NH + NH], F32, tag="ae")
        for t in range(T):
            nc.tensor.matmul(ae[:, t * NH:(t + 1) * NH],
                             ST_sb[:, t * 128:(t + 1) * 128], adst_sb[:],
                             start=True, stop=True)
        ep = sbs.tile([128, T * NH], BF16, tag="ep")
        nc.vector.tensor_tensor(ep[:], ae[:, 0:T * NH],
                                gat[:, :, ACOL:ACOL + NH], OP.add)
        lr = sbs.tile([128, T * NH], BF16, tag="lr")
        nc.vector.scalar_tensor_tensor(lr[:], ep[:], NEG, ep[:], OP.mult, OP.max)
        p = sbs.tile([128, T * NH], F32, tag="p")
        nc.scalar.activation(p[:], lr[:], AF.Exp)
        p16 = sbs.tile([128, T * NH], BF16, tag="p16")
        nc.scalar.activation(p16[:], p[:], AF.Copy)

        oacc = psb.tile([128, FEAT], F32, tag="oacc")
        dacc = ae[:, T * NH:T * NH + NH]
        for t in range(T):
            w = sbw.tile([128, FEAT], BF16, tag="w")
            if lay == 1:
                for h in range(2):
                    nc.scalar.activation(
                        w[:, h * HID:(h + 1) * HID], gat[:, t, h * HID:(h + 1) * HID],
                        AF.Copy, scale=p[:, t * NH + h:t * NH + h + 1])
                for h in range(2, 4):
                    nc.vector.tensor_scalar_mul(
                        w[:, h * HID:(h + 1) * HID], gat[:, t, h * HID:(h + 1) * HID],
                        p[:, t * NH + h:t * NH + h + 1])
            else:
                if t % 2 == 0:
                    nc.scalar.activation(w[:], gat[:, t, 0:FEAT], AF.Copy,
                                         scale=p[:, t:t + 1])
                else:
                    nc.vector.tensor_scalar_mul(w[:], gat[:, t, 0:FEAT],
                                                p[:, t:t + 1])
            nc.tensor.matmul(oacc[:], S_sb[:, t * 128:(t + 1) * 128], w[:],
                             start=(t == 0), stop=(t == T - 1))
            nc.tensor.matmul(dacc, S_sb[:, t * 128:(t + 1) * 128],
                             p16[:, t * NH:(t + 1) * NH],
                             start=(t == 0), stop=(t == T - 1))

        out_writer(b, base, bs, oacc, dacc)


def _build(meta):
    nc = bacc.Bacc("TRN2", target_bir_lowering=False, debug=False, num_devices=C)
    TT = meta["TT"]

    xT_d = nc.dram_tensor("xT", [F_IN, NS], BF16, kind="ExternalInput").ap()
    rhs1_d = nc.dram_tensor("rhs1", [F_IN, 520], BF16, kind="ExternalInput").ap()
    rhs2_d = nc.dram_tensor("rhs2", [F_IN, 130], BF16, kind="ExternalInput").ap()
    b1r_d = nc.dram_tensor("b1r", [128, 512], F32, kind="ExternalInput").ap()
    b2r_d = nc.dram_tensor("b2r", [128, 128], F32, kind="ExternalInput").ap()
    ident_d = nc.dram_tensor("ident", [128, 128], BF16, kind="ExternalInput").ap()
    idx_d = nc.dram_tensor("idx", [128, TT * 8], I16, kind="ExternalInput").ap()
    S_d = nc.dram_tensor("S", [128, TT * 128], BF16, kind="ExternalInput").ap()
    ST_d = nc.dram_tensor("ST", [128, TT * 128], BF16, kind="ExternalInput").ap()
    out_d = nc.dram_tensor("out", [NS, HID], F32, kind="ExternalOutput").ap()

    meta = dict(meta)
    meta["idx_ap"], meta["S_ap"], meta["ST_ap"] = idx_d, S_d, ST_d

    with tile.TileContext(nc, num_cores=C) as tc:
        with tc.tile_pool(name="dram", bufs=1, space="DRAM") as dram:
            hb1 = dram.tile([NS, ROW1], BF16)
            hfull1 = dram.tile([N, ROW1], BF16, addr_space="Shared")
            ad1 = dram.tile([NS, H1], BF16)
            hb2 = dram.tile([NS, ROW2], BF16)
            hfull2 = dram.tile([N, ROW2], BF16, addr_space="Shared")
            ad2 = dram.tile([NS, 1], BF16)

            # ---------------- phase A: h1 shard + attn scalars ----------------
            with (
                tc.tile_pool(name="a_c", bufs=1) as sbc,
                tc.tile_pool(name="a_w", bufs=3) as sbw,
                tc.tile_pool(name="a_p", bufs=2, space="PSUM") as psp,
            ):
                rhs1_sb = []
                for k in range(4):
                    rt = sbc.tile([128, 520], BF16, name=f"rhs1sb{k}")
                    nc.sync.dma_start(rt[:], rhs1_d[k * 128:(k + 1) * 128, :])
                    rhs1_sb.append(rt)
                for b in range(NBLK):
                    bs = CAPS[b]
                    base = b * 128
                    ph = psp.tile([128, 512], F32, tag="ph")
                    pa = psp.tile([128, 8], F32, tag="pa")
                    for k in range(4):
                        xt = sbw.tile([128, 128], BF16, tag="xt")
                        nc.sync.dma_start(xt[:, :bs], xT_d[k * 128:(k + 1) * 128, base:base + bs])
                        nc.tensor.matmul(ph[:bs, :], xt[:, :bs], rhs1_sb[k][:, 0:512],
                                         start=(k == 0), stop=(k == 3))
                        nc.tensor.matmul(pa[:bs, :], xt[:, :bs], rhs1_sb[k][:, 512:520],
                                         start=(k == 0), stop=(k == 3))
                    ha = sbw.tile([128, ROW1], BF16, tag="ha")
                    nc.scalar.activation(ha[:bs, 0:512], ph[:bs, :], AF.Copy)
                    nc.scalar.activation(ha[:bs, 512:516], pa[:bs, 0:4], AF.Copy)
                    nc.vector.memset(ha[:bs, 516:ROW1], 0.0)
                    nc.sync.dma_start(hb1[base:base + bs, :], ha[:bs, :])
                    adsb = sbw.tile([128, H1], BF16, tag="adsb")
                    nc.scalar.activation(adsb[:bs, :], pa[:bs, 4:8], AF.Copy)
                    nc.sync.dma_start(ad1[base:base + bs, :], adsb[:bs, :])

            nc.gpsimd.collective_compute(
                "AllGather", OP.bypass, replica_groups=[list(range(C))],
                ins=[hb1[:].opt()], outs=[hfull1[:].opt()])

            # ---------------- phase B: layer-1 edge pass + h2@W2 ----------------
            with (
                tc.tile_pool(name="b_c", bufs=1) as sbc,
                tc.tile_pool(name="b_m", bufs=2) as sbm,
                tc.tile_pool(name="b_g", bufs=2) as sbg,
                tc.tile_pool(name="b_s", bufs=2) as sbs,
                tc.tile_pool(name="b_w", bufs=4) as sbw,
                tc.tile_pool(name="b_w2", bufs=2) as sbw2,
                tc.tile_pool(name="b_pb", bufs=2, space="PSUM") as psb,
                tc.tile_pool(name="b_pa", bufs=2, space="PSUM") as psa,
                tc.tile_pool(name="b_ph", bufs=1, space="PSUM") as psh,
                tc.tile_pool(name="b_pt", bufs=2, space="PSUM") as pst,
            ):
                b1r_sb = sbc.tile([128, 512], F32, name="b1rsb")
                nc.sync.dma_start(b1r_sb[:], b1r_d)
                ident_sb = sbc.tile([128, 128], BF16, name="identsb")
                nc.sync.dma_start(ident_sb[:], ident_d)
                rhs2_sb = []
                for k in range(4):
                    rt = sbc.tile([128, 130], BF16, name=f"rhs2sb{k}")
                    nc.sync.dma_start(rt[:], rhs2_d[k * 128:(k + 1) * 128, :])
                    rhs2_sb.append(rt)

                def writer_b(b, base, bs, oacc, dacc):
                    den = sbw2.tile([128, 4], F32, tag="den")
                    nc.vector.tensor_scalar_add(den[:], dacc, EPS)
                    rec = sbw2.tile([128, 4], F32, tag="rec")
                    nc.vector.reciprocal(rec[:], den[:])
                    h2b = sbw2.tile([128, 512], BF16, tag="h2b")
                    for h in range(4):
                        nc.vector.scalar_tensor_tensor(
                            h2b[:, h * HID:(h + 1) * HID], oacc[:, h * HID:(h + 1) * HID],
                            rec[:, h:h + 1], b1r_sb[:, h * HID:(h + 1) * HID],
                            OP.mult, OP.add)
                    rl = sbw2.tile([128, 512], BF16, tag="rl")
                    nc.scalar.activation(rl[:], h2b[:], AF.Relu)
                    mn = sbw2.tile([128, 512], BF16, tag="mn")
                    nc.vector.tensor_scalar_min(mn[:], h2b[:], 0.0)
                    em = sbw2.tile([128, 512], BF16, tag="em")
                    nc.scalar.activation(em[:], mn[:], AF.Exp)
                    h2f = sbw2.tile([128, 512], BF16, tag="h2f")
                    nc.vector.scalar_tensor_tensor(h2f[:], em[:], -1.0, rl[:], OP.add, OP.add)
                    hh = psh.tile([128, 130], F32, tag="hh")
                    for k in range(4):
                        tp = pst.tile([128, 128], BF16, tag="tp")
                        nc.tensor.transpose(tp[:], h2f[:, k * 128:(k + 1) * 128], ident_sb[:])
                        h2T = sbw2.tile([128, 128], BF16, tag="h2T")
                        nc.scalar.activation(h2T[:], tp[:], AF.Copy)
                        nc.tensor.matmul(hh[:], h2T[:], rhs2_sb[k][:], start=(k == 0), stop=(k == 3))
                    ha2 = sbw2.tile([128, ROW2], BF16, tag="ha2")
                    nc.scalar.activation(ha2[:bs, 0:129], hh[:bs, 0:129], AF.Copy)
                    nc.vector.memset(ha2[:bs, 129:ROW2], 0.0)
                    nc.sync.dma_start(hb2[base:base + bs, :], ha2[:bs, :])
                    adsb2 = sbw2.tile([128, 1], BF16, tag="adsb2")
                    nc.scalar.activation(adsb2[:bs, :], hh[:bs, 129:130], AF.Copy)
                    nc.sync.dma_start(ad2[base:base + bs, :], adsb2[:bs, :])

                _edge_layer(nc, tc, meta, 1, (sbm, sbg, sbs, sbw, psb, psa),
                            ad1, hfull1, writer_b)

            nc.gpsimd.collective_compute(
                "AllGather", OP.bypass, replica_groups=[list(range(C))],
                ins=[hb2[:].opt()], outs=[hfull2[:].opt()])

            # ---------------- phase D: layer-2 edge pass ----------------
            with (
                tc.tile_pool(name="d_c", bufs=1) as sbc,
                tc.tile_pool(name="d_m", bufs=2) as sbm,
                tc.tile_pool(name="d_g", bufs=2) as sbg,
                tc.tile_pool(name="d_s", bufs=2) as sbs,
                tc.tile_pool(name="d_w", bufs=4) as sbw,
                tc.tile_pool(name="d_w2", bufs=2) as sbw2,
                tc.tile_pool(name="d_pb", bufs=2, space="PSUM") as psb,
                tc.tile_pool(name="d_pa", bufs=2, space="PSUM") as psa,
            ):
                b2r_sb = sbc.tile([128, 128], F32, name="b2rsb")
                nc.sync.dma_start(b2r_sb[:], b2r_d)

                def writer_d(b, base, bs, oacc, dacc):
                    den = sbw2.tile([128, 1], F32, tag="den")
                    nc.vector.tensor_scalar_add(den[:], dacc, EPS)
                    rec = sbw2.tile([128, 1], F32, tag="rec")
                    nc.vector.reciprocal(rec[:], den[:])
                    ofb = sbw2.tile([128, 128], F32, tag="ofb")
                    nc.vector.scalar_tensor_tensor(ofb[:], oacc[:], rec[:, 0:1],
                                                   b2r_sb[:], OP.mult, OP.add)
                    nc.sync.dma_start(out_d[base:base + bs, :], ofb[:bs, :])

                _edge_layer(nc, tc, meta, 2, (sbm, sbg, sbs, sbw, psb, psa),
                            ad2, hfull2, writer_d)

    nc.compile()
    return nc


# --------------------------------------------------------------------------
# entry point
# --------------------------------------------------------------------------

def kernel(x, edge_index, W1, a_src1, a_dst1, b1, W2, a_src2, a_dst2, b2,
           _trace=False):
    in_maps, meta, perm_order = _prepare(
        x, edge_index, W1, a_src1, a_dst1, b1, W2, a_src2, a_dst2, b2)

    import time as _time
    _t0 = _time.time()
    key = (meta["TT"], tuple(meta["T_lo"]), tuple(meta["T_hi"]))
    if key not in _cache:
        _cache.clear()
        _cache[key] = _build(meta)
    nc = _cache[key]
    print(f"[kernel] build done at {_time.time()-_t0:.1f}s", flush=True)

    kw = {}
    if _trace:
        kw = dict(trace=True)
    res = bass_utils.run_bass_kernel_spmd(nc, in_maps, core_ids=list(range(C)), **kw)

    out = np.empty((N, HID), np.float32)
    for c in range(C):
        out[perm_order[c * NS:(c + 1) * NS]] = res.results[c]["out"]
    kernel._last_result = res
    return out


# revision 6
# speedup vs baseline: 1.2805x; 1.2805x over previous
"""Trainium2 Bass kernel for nn_ClassDiagramGNN: 2-layer GAT on 50k nodes / 850k edges.

Strategy (8 NeuronCores, dst-sharded graph parallel):
  - Host: add self-loops, global LPT of nodes onto cores by degree, per-core
    LPT into 128-wide dst blocks, permute node ids so each block is
    contiguous. Per block, the 128 self-loop edges form a dedicated tile
    whose source rows are a contiguous slice of the local shard (direct DMA,
    no gather). Remaining edges are bucketed by (block, src-half) and packed
    into 128-edge tiles; trailing pad slots get idx=-1 so the gather skips
    them. The one-hot edge->dst scatter matrix S and its transpose ST are
    precomputed per tile in bf16 and streamed in by DMA.
  - Phase A: resident xT; per block h1 = x @ W1 plus attention scalars
    (folded into the matmul), packed as 640-col bf16 rows -> AllGather.
    Per-node adst scalars also go to a compact local table.
  - Edge pass (both layers), software-pipelined per block: prefetch DMAs +
    gathers for block b+1 while computing block b. Attention: ae = ST @ adst
    per tile accumulated into one PSUM strip, then batched add/leaky/exp over
    [128, T*NH]; per-tile w = p * h (split across DVE and ACT), aggregate via
    bf16 S^T @ w matmuls in PSUM, denominator S^T @ p, normalize, ELU,
    h2 @ W2 -> 256-col bf16 rows -> AllGather -> layer-2 pass -> fp32 out.
"""
import sys

for _p in ("/opt/trn_rl_repo",):
    if _p not in sys.path:
        sys.path.append(_p)

import heapq
import numpy as np
import ml_dtypes

import concourse.bass as bass
import concourse.bacc as bacc
import concourse.tile as tile
from concourse import mybir
from concourse import bass_utils

F32 = mybir.dt.float32
BF16 = mybir.dt.bfloat16
I16 = mybir.dt.int16
AF = mybir.ActivationFunctionType
OP = mybir.AluOpType
NPBF = ml_dtypes.bfloat16

# problem constants (hardcoded per contract)
N, F_IN, HID, H1, E = 50000, 512, 128, 4, 800000
NEG = 0.2
C = 8                 # cores
NS = N // C           # 6250 nodes per shard
NBLK = (NS + 127) // 128   # 49 blocks per core
CAPS = [128] * (NBLK - 1) + [NS - 128 * (NBLK - 1)]  # 48x128 + 106
HALF = N // 2         # gather-table half split (int16 index reach)
ROW1 = 640            # layer-1 bf16 row: 512 feat + 4 asrc + pad (1280B)
ROW2 = 256            # layer-2 bf16 row: 128 feat + 1 asrc + pad (512B)
EPS = 1e-16

_cache = {}


def _reconfigure(n, e):
    """Testing hook: shrink the graph (keeps F_IN/HID/H1 fixed)."""
    global N, E, NS, NBLK, CAPS, HALF
    N, E = n, e
    NS = N // C
    NBLK = (NS + 127) // 128
    CAPS = [128] * (NBLK - 1) + [NS - 128 * (NBLK - 1)]
    HALF = N // 2
    _cache.clear()


# --------------------------------------------------------------------------
# host-side preprocessing
# --------------------------------------------------------------------------

def _prepare(x, edge_index, W1, a_src1, a_dst1, b1, W2, a_src2, a_dst2, b2):
    # self-loops handled as dedicated per-block tiles; bucket only real edges
    src = edge_index[0].astype(np.int64)
    dst = edge_index[1].astype(np.int64)
    deg = np.bincount(dst, minlength=N) + 1  # +1 self loop

    # global LPT of nodes onto cores by degree (balances edge counts), then
    # per-core LPT into blocks
    perm_pos = np.empty(N, dtype=np.int64)    # orig id -> permuted global pos
    perm_order = np.empty(N, dtype=np.int64)  # permuted pos -> orig id
    order_g = np.argsort(-deg, kind="stable")
    cheap = [(0, 0, ci) for ci in range(C)]
    heapq.heapify(cheap)
    core_nodes = [[] for _ in range(C)]
    for nid in order_g:
        while True:
            load, used, ci = heapq.heappop(cheap)
            if used < NS:
                break
        core_nodes[ci].append(nid)
        heapq.heappush(cheap, (load + int(deg[nid]), used + 1, ci))
    for c in range(C):
        ids = np.array(core_nodes[c])
        d = deg[ids]
        order = np.argsort(-d, kind="stable")
        heap = [(0, 0, i) for i in range(NBLK)]
        heapq.heapify(heap)
        assign = [[] for _ in range(NBLK)]
        for li in order:
            while True:
                load, used, bi = heapq.heappop(heap)
                if used < CAPS[bi]:
                    break
            assign[bi].append(li)
            heapq.heappush(heap, (load + int(d[li]), used + 1, bi))
        pos = 0
        for bi in range(NBLK):
            for li in assign[bi]:
                g = c * NS + pos
                perm_pos[ids[li]] = g
                perm_order[g] = ids[li]
                pos += 1

    src_p = perm_pos[src]
    dst_p = perm_pos[dst]
    core = dst_p // NS
    blk = (dst_p % NS) // 128
    halfv = (src_p >= HALF).astype(np.int64)
    key = (core * NBLK + blk) * 2 + halfv
    eorder = np.argsort(key, kind="stable")
    counts = np.bincount(key, minlength=C * NBLK * 2).reshape(C, NBLK, 2)

    # cross-core uniform tile counts per (block, half); +1 self tile per block
    T_lo = -(-counts[:, :, 0].max(axis=0) // 128)  # ceil
    T_hi = -(-counts[:, :, 1].max(axis=0) // 128)
    T_all = 1 + T_lo + T_hi                        # self tile first
    TT = int(T_all.sum())
    Tmax = int(T_all.max())
    toff = np.zeros(NBLK, np.int64)
    toff[1:] = np.cumsum(T_all)[:-1]

    src_sorted = src_p[eorder]
    dloc_sorted = (dst_p[eorder] % NS) % 128
    starts = np.zeros(C * NBLK * 2 + 1, np.int64)
    starts[1:] = np.cumsum(counts.reshape(-1))

    idx_all = np.zeros((C, TT * 128), np.int16)           # pad -> row 0
    dc_all = np.full((C, TT * 128), 999, np.int64)        # pad -> no dst
    nreal = np.zeros((C, NBLK, 2), np.int64)
    for c in range(C):
        for b in range(NBLK):
            # self tile: dst-local row i <- node base+i (contiguous source)
            bs = CAPS[b]
            slot0 = toff[b] * 128
            dc_all[c, slot0:slot0 + bs] = np.arange(bs)
            for h in range(2):
                k = (c * NBLK + b) * 2 + h
                s0, s1 = starts[k], starts[k + 1]
                n = s1 - s0
                nreal[c, b, h] = n
                if n == 0:
                    continue
                slot0 = (toff[b] + 1 + (T_lo[b] if h else 0)) * 128
                seg = src_sorted[s0:s1]
                if h:
                    seg = seg - HALF
                idx_all[c, slot0:slot0 + n] = seg.astype(np.int16)
                dc_all[c, slot0:slot0 + n] = dloc_sorted[s0:s1]

    # full-range gathers (negative-index skipping crashes the gather ucode,
    # so pad slots gather row 0 instead)
    g_lo = T_lo * 128
    g_hi = T_hi * 128

    # weights: fold per-head attention projections into the linear transforms
    W1_64 = np.asarray(W1, np.float64)
    Dsrc1 = np.zeros((H1 * HID, H1), np.float64)
    Ddst1 = np.zeros((H1 * HID, H1), np.float64)
    a_src1_64 = np.asarray(a_src1, np.float64)
    a_dst1_64 = np.asarray(a_dst1, np.float64)
    for h in range(H1):
        Dsrc1[h * HID:(h + 1) * HID, h] = a_src1_64[h]
        Ddst1[h * HID:(h + 1) * HID, h] = a_dst1_64[h]
    rhs1 = np.concatenate(
        [np.asarray(W1, np.float32),
         (W1_64 @ Dsrc1).astype(np.float32),
         (W1_64 @ Ddst1).astype(np.float32)], axis=1).astype(NPBF)  # [512, 520]
    W2_64 = np.asarray(W2, np.float64)
    rhs2 = np.concatenate(
        [np.asarray(W2, np.float32),
         (W2_64 @ np.asarray(a_src2, np.float64)[0][:, None]).astype(np.float32),
         (W2_64 @ np.asarray(a_dst2, np.float64)[0][:, None]).astype(np.float32)],
        axis=1).astype(NPBF)                                        # [512, 130]

    ident = np.eye(128, dtype=NPBF)
    b1r = np.tile(np.asarray(b1, np.float32)[None, :], (128, 1))
    b2r = np.tile(np.asarray(b2, np.float32)[None, :], (128, 1))

    slots = np.arange(TT * 128)
    tloc = slots // 128
    posi = slots % 128

    xnp = np.asarray(x, np.float32)
    in_maps = []
    for c in range(C):
        rows = perm_order[c * NS:(c + 1) * NS]
        xT = np.ascontiguousarray(xnp[rows].T).astype(NPBF)     # [512, 6250]
        idx_w = np.tile(idx_all[c].reshape(-1, 16).T, (8, 1))   # [128, TT*8]
        dcv = dc_all[c]
        valid = dcv < 128
        dv = dcv[valid]
        tv = tloc[valid]
        pv = posi[valid]
        S_all = np.zeros((128, TT * 128), NPBF)
        S_all[pv, tv * 128 + dv] = 1
        ST_all = np.zeros((128, TT * 128), NPBF)
        ST_all[dv, tv * 128 + pv] = 1
        in_maps.append({
            "xT": xT, "rhs1": rhs1, "rhs2": rhs2,
            "b1r": b1r, "b2r": b2r, "ident": ident,
            "idx": np.ascontiguousarray(idx_w),
            "S": S_all, "ST": ST_all,
        })

    meta = {
        "T_lo": [int(v) for v in T_lo],
        "T_hi": [int(v) for v in T_hi],
        "toff": [int(v) for v in toff],
        "g_lo": [int(v) for v in g_lo],
        "g_hi": [int(v) for v in g_hi],
        "TT": TT,
        "Tmax": Tmax,
    }
    return in_maps, meta, perm_order


# --------------------------------------------------------------------------
# device program
# --------------------------------------------------------------------------

def _edge_layer(nc, tc, meta, lay, pools, ad_my, hb_my, hfull, out_writer):
    """Software-pipelined per-block edge pass shared by both GAT layers.

    lay=1: ROW=640, 4 heads, feat cols 0:512, asrc 512:516
    lay=2: ROW=256, 1 head, feat cols 0:128, asrc 128:129
    """
    sbm, sbg, sbs, sbw, psb, psa = pools
    ROW = ROW1 if lay == 1 else ROW2
    NH = H1 if lay == 1 else 1
    FEAT = NH * HID
    ACOL = FEAT
    idx_d, S_d, ST_d = meta["idx_ap"], meta["S_ap"], meta["ST_ap"]
    Tmax = meta["Tmax"]

    # one-time zero of the gather-pool buffers so skipped pad slots never
    # expose uninitialized SBUF (NaN/Inf) to the attention math
    for _ in range(sbg.bufs):
        gz = sbg.tile([128, Tmax, ROW], BF16, tag="gat")
        nc.vector.memset(gz[:, :, :], 0.0)

    def prefetch(b):
        bs = CAPS[b]
        base = b * 128
        T_lo, T_hi = meta["T_lo"][b], meta["T_hi"][b]
        T = 1 + T_lo + T_hi
        boff = meta["toff"][b]
        g_lo, g_hi = meta["g_lo"][b], meta["g_hi"][b]

        idx_sb = sbm.tile([128, Tmax * 8], I16, tag="idx")
        nc.sync.dma_start(idx_sb[:, 0:T * 8], idx_d[:, boff * 8:(boff + T) * 8])
        S_sb = sbm.tile([128, Tmax * 128], BF16, tag="S")
        nc.sync.dma_start(S_sb[:, 0:T * 128], S_d[:, boff * 128:(boff + T) * 128])
        ST_sb = sbm.tile([128, Tmax * 128], BF16, tag="ST")
        nc.sync.dma_start(ST_sb[:, 0:T * 128], ST_d[:, boff * 128:(boff + T) * 128])
        adst_sb = sbm.tile([128, NH], BF16, tag="adst")
        if bs < 128:
            nc.vector.memset(adst_sb[:], 0.0)
        nc.sync.dma_start(adst_sb[:bs], ad_my[base:base + bs, :])

        gat = sbg.tile([128, Tmax, ROW], BF16, tag="gat")
        # self tile: contiguous local rows
        nc.sync.dma_start(gat[:bs, 0, :], hb_my[base:base + bs, :])
        if g_lo:
            nc.gpsimd.dma_gather(
                gat[:, 1:1 + g_lo // 128, :], hfull[0:HALF, :],
                idx_sb[:, 8:(1 + g_lo // 128) * 8],
                g_lo, g_lo, ROW, elem_step=ROW, single_packet=False)
        if g_hi:
            t0 = 1 + T_lo
            nc.gpsimd.dma_gather(
                gat[:, t0:t0 + g_hi // 128, :], hfull[HALF:N, :],
                idx_sb[:, t0 * 8:(t0 + g_hi // 128) * 8],
                g_hi, g_hi, ROW, elem_step=ROW, single_packet=False)
        return (b, bs, base, T, idx_sb, S_sb, ST_sb, adst_sb, gat)

    def compute(st):
        b, bs, base, T, idx_sb, S_sb, ST_sb, adst_sb, gat = st
        ae = psa.tile([128, Tmax * NH + NH], F32, tag="ae")
        for t in range(T):
            nc.tensor.matmul(ae[:, t * NH:(t + 1) * NH],
                             ST_sb[:, t * 128:(t + 1) * 128], adst_sb[:],
                             start=True, stop=True)
        ep = sbs.tile([128, Tmax * NH], BF16, tag="ep")
        nc.vector.tensor_tensor(ep[:, 0:T * NH], ae[:, 0:T * NH],
                                gat[:, 0:T, ACOL:ACOL + NH], OP.add)
        lr = sbs.tile([128, Tmax * NH], BF16, tag="lr")
        nc.vector.scalar_tensor_tensor(lr[:, 0:T * NH], ep[:, 0:T * NH],
                                       NEG, ep[:, 0:T * NH], OP.mult, OP.max)
        p = sbs.tile([128, Tmax * NH], F32, tag="p")
        nc.scalar.activation(p[:, 0:T * NH], lr[:, 0:T * NH], AF.Exp)
        p16 = sbs.tile([128, Tmax * NH], BF16, tag="p16")
        nc.scalar.activation(p16[:, 0:T * NH], p[:, 0:T * NH], AF.Copy)

        oacc = psb.tile([128, FEAT], F32, tag="oacc")
        dacc = ae[:, Tmax * NH:Tmax * NH + NH]
        for t in range(T):
            w = sbw.tile([128, FEAT], BF16, tag="w")
            if lay == 1:
                nc.scalar.activation(w[:, 0:HID], gat[:, t, 0:HID],
                                     AF.Copy, scale=p[:, t * NH:t * NH + 1])
                for h in range(1, 4):
                    nc.vector.tensor_scalar_mul(
                        w[:, h * HID:(h + 1) * HID], gat[:, t, h * HID:(h + 1) * HID],
                        p[:, t * NH + h:t * NH + h + 1])
            else:
                if t % 2 == 0:
                    nc.scalar.activation(w[:], gat[:, t, 0:FEAT], AF.Copy,
                                         scale=p[:, t:t + 1])
                else:
                    nc.vector.tensor_scalar_mul(w[:], gat[:, t, 0:FEAT],
                                                p[:, t:t + 1])
            nc.tensor.matmul(oacc[:], S_sb[:, t * 128:(t + 1) * 128], w[:],
                             start=(t == 0), stop=(t == T - 1))
            nc.tensor.matmul(dacc, S_sb[:, t * 128:(t + 1) * 128],
                             p16[:, t * NH:(t + 1) * NH],
                             start=(t == 0), stop=(t == T - 1))
        out_writer(b, base, bs, oacc, dacc)

    st = prefetch(0)
    for b in range(NBLK):
        nxt = prefetch(b + 1) if b + 1 < NBLK else None
        compute(st)
        st = nxt


def _build(meta):
    nc = bacc.Bacc("TRN2", target_bir_lowering=False, debug=False, num_devices=C)
    TT = meta["TT"]

    xT_d = nc.dram_tensor("xT", [F_IN, NS], BF16, kind="ExternalInput").ap()
    rhs1_d = nc.dram_tensor("rhs1", [F_IN, 520], BF16, kind="ExternalInput").ap()
    rhs2_d = nc.dram_tensor("rhs2", [F_IN, 130], BF16, kind="ExternalInput").ap()
    b1r_d = nc.dram_tensor("b1r", [128, 512], F32, kind="ExternalInput").ap()
    b2r_d = nc.dram_tensor("b2r", [128, 128], F32, kind="ExternalInput").ap()
    ident_d = nc.dram_tensor("ident", [128, 128], BF16, kind="ExternalInput").ap()
    idx_d = nc.dram_tensor("idx", [128, TT * 8], I16, kind="ExternalInput").ap()
    S_d = nc.dram_tensor("S", [128, TT * 128], BF16, kind="ExternalInput").ap()
    ST_d = nc.dram_tensor("ST", [128, TT * 128], BF16, kind="ExternalInput").ap()
    out_d = nc.dram_tensor("out", [NS, HID], F32, kind="ExternalOutput").ap()

    meta = dict(meta)
    meta["idx_ap"], meta["S_ap"], meta["ST_ap"] = idx_d, S_d, ST_d

    with tile.TileContext(nc, num_cores=C) as tc:
        with tc.tile_pool(name="dram", bufs=1, space="DRAM") as dram:
            hb1 = dram.tile([NS, ROW1], BF16)
            hfull1 = dram.tile([N, ROW1], BF16, addr_space="Shared")
            ad1 = dram.tile([NS, H1], BF16)
            hb2 = dram.tile([NS, ROW2], BF16)
            hfull2 = dram.tile([N, ROW2], BF16, addr_space="Shared")
            ad2 = dram.tile([NS, 1], BF16)

            # ---------------- phase A: h1 shard + attn scalars ----------------
            with (
                tc.tile_pool(name="a_c", bufs=1) as sbc,
                tc.tile_pool(name="a_w", bufs=3) as sbw,
                tc.tile_pool(name="a_p", bufs=2, space="PSUM") as psp,
            ):
                rhs1_sb = []
                xt_sb = []
                for k in range(4):
                    rt = sbc.tile([128, 520], BF16, name=f"rhs1sb{k}")
                    nc.sync.dma_start(rt[:], rhs1_d[k * 128:(k + 1) * 128, :])
                    rhs1_sb.append(rt)
                    xt = sbc.tile([128, NS], BF16, name=f"xtsb{k}")
                    nc.sync.dma_start(xt[:], xT_d[k * 128:(k + 1) * 128, :])
                    xt_sb.append(xt)
                for b in range(NBLK):
                    bs = CAPS[b]
                    base = b * 128
                    ph = psp.tile([128, 512], F32, tag="ph")
                    pa = psp.tile([128, 8], F32, tag="pa")
                    for k in range(4):
                        nc.tensor.matmul(ph[:bs, :], xt_sb[k][:, base:base + bs],
                                         rhs1_sb[k][:, 0:512],
                                         start=(k == 0), stop=(k == 3))
                        nc.tensor.matmul(pa[:bs, :], xt_sb[k][:, base:base + bs],
                                         rhs1_sb[k][:, 512:520],
                                         start=(k == 0), stop=(k == 3))
                    ha = sbw.tile([128, ROW1], BF16, tag="ha")
                    nc.scalar.activation(ha[:bs, 0:512], ph[:bs, :], AF.Copy)
                    nc.scalar.activation(ha[:bs, 512:516], pa[:bs, 0:4], AF.Copy)
                    nc.vector.memset(ha[:bs, 516:ROW1], 0.0)
                    nc.sync.dma_start(hb1[base:base + bs, :], ha[:bs, :])
                    adsb = sbw.tile([128, H1], BF16, tag="adsb")
                    nc.scalar.activation(adsb[:bs, :], pa[:bs, 4:8], AF.Copy)
                    nc.sync.dma_start(ad1[base:base + bs, :], adsb[:bs, :])

            nc.gpsimd.collective_compute(
                "AllGather", OP.bypass, replica_groups=[list(range(C))],
                ins=[hb1[:].opt()], outs=[hfull1[:].opt()])

            # ---------------- phase B: layer-1 edge pass + h2@W2 ----------------
            with (
                tc.tile_pool(name="b_c", bufs=1) as sbc,
                tc.tile_pool(name="b_m", bufs=2) as sbm,
                tc.tile_pool(name="b_g", bufs=2) as sbg,
                tc.tile_pool(name="b_s", bufs=2) as sbs,
                tc.tile_pool(name="b_w", bufs=4) as sbw,
                tc.tile_pool(name="b_w2", bufs=2) as sbw2,
                tc.tile_pool(name="b_pb", bufs=2, space="PSUM") as psb,
                tc.tile_pool(name="b_pa", bufs=2, space="PSUM") as psa,
                tc.tile_pool(name="b_ph", bufs=1, space="PSUM") as psh,
                tc.tile_pool(name="b_pt", bufs=2, space="PSUM") as pst,
            ):
                b1r_sb = sbc.tile([128, 512], F32, name="b1rsb")
                nc.sync.dma_start(b1r_sb[:], b1r_d)
                ident_sb = sbc.tile([128, 128], BF16, name="identsb")
                nc.sync.dma_start(ident_sb[:], ident_d)
                rhs2_sb = []
                for k in range(4):
                    rt = sbc.tile([128, 130], BF16, name=f"rhs2sb{k}")
                    nc.sync.dma_start(rt[:], rhs2_d[k * 128:(k + 1) * 128, :])
                    rhs2_sb.append(rt)

                def writer_b(b, base, bs, oacc, dacc):
                    den = sbw2.tile([128, 4], F32, tag="den")
                    nc.vector.tensor_scalar_add(den[:], dacc, EPS)
                    rec = sbw2.tile([128, 4], F32, tag="rec")
                    nc.vector.reciprocal(rec[:], den[:])
                    h2b = sbw2.tile([128, 512], BF16, tag="h2b")
                    for h in range(4):
                        nc.vector.scalar_tensor_tensor(
                            h2b[:, h * HID:(h + 1) * HID], oacc[:, h * HID:(h + 1) * HID],
                            rec[:, h:h + 1], b1r_sb[:, h * HID:(h + 1) * HID],
                            OP.mult, OP.add)
                    rl = sbw2.tile([128, 512], BF16, tag="rl")
                    nc.scalar.activation(rl[:], h2b[:], AF.Relu)
                    mn = sbw2.tile([128, 512], BF16, tag="mn")
                    nc.vector.tensor_scalar_min(mn[:], h2b[:], 0.0)
                    em = sbw2.tile([128, 512], BF16, tag="em")
                    nc.scalar.activation(em[:], mn[:], AF.Exp)
                    h2f = sbw2.tile([128, 512], BF16, tag="h2f")
                    nc.vector.scalar_tensor_tensor(h2f[:], em[:], -1.0, rl[:], OP.add, OP.add)
                    hh = psh.tile([128, 130], F32, tag="hh")
                    for k in range(4):
                        tp = pst.tile([128, 128], BF16, tag="tp")
                        nc.tensor.transpose(tp[:], h2f[:, k * 128:(k + 1) * 128], ident_sb[:])
                        h2T = sbw2.tile([128, 128], BF16, tag="h2T")
                        nc.vector.tensor_copy(h2T[:], tp[:])
                        nc.tensor.matmul(hh[:], h2T[:], rhs2_sb[k][:], start=(k == 0), stop=(k == 3))
                    ha2 = sbw2.tile([128, ROW2], BF16, tag="ha2")
                    nc.scalar.activation(ha2[:bs, 0:129], hh[:bs, 0:129], AF.Copy)
                    nc.vector.memset(ha2[:bs, 129:ROW2], 0.0)
                    nc.sync.dma_start(hb2[base:base + bs, :], ha2[:bs, :])
                    adsb2 = sbw2.tile([128, 1], BF16, tag="adsb2")
                    nc.scalar.activation(adsb2[:bs, :], hh[:bs, 129:130], AF.Copy)
                    nc.sync.dma_start(ad2[base:base + bs, :], adsb2[:bs, :])

                _edge_layer(nc, tc, meta, 1, (sbm, sbg, sbs, sbw, psb, psa),
                            ad1, hb1, hfull1, writer_b)

            nc.gpsimd.collective_compute(
                "AllGather", OP.bypass, replica_groups=[list(range(C))],
                ins=[hb2[:].opt()], outs=[hfull2[:].opt()])

            # ---------------- phase D: layer-2 edge pass ----------------
            with (
                tc.tile_pool(name="d_c", bufs=1) as sbc,
                tc.tile_pool(name="d_m", bufs=2) as sbm,
                tc.tile_pool(name="d_g", bufs=2) as sbg,
                tc.tile_pool(name="d_s", bufs=2) as sbs,
                tc.tile_pool(name="d_w", bufs=4) as sbw,
                tc.tile_pool(name="d_w2", bufs=2) as sbw2,
                tc.tile_pool(name="d_pb", bufs=2, space="PSUM") as psb,
                tc.tile_pool(name="d_pa", bufs=2, space="PSUM") as psa,
            ):
                b2r_sb = sbc.tile([128, 128], F32, name="b2rsb")
                nc.sync.dma_start(b2r_sb[:], b2r_d)

                def writer_d(b, base, bs, oacc, dacc):
                    den = sbw2.tile([128, 1], F32, tag="den")
                    nc.vector.tensor_scalar_add(den[:], dacc, EPS)
                    rec = sbw2.tile([128, 1], F32, tag="rec")
                    nc.vector.reciprocal(rec[:], den[:])
                    ofb = sbw2.tile([128, 128], F32, tag="ofb")
                    nc.vector.scalar_tensor_tensor(ofb[:], oacc[:], rec[:, 0:1],
                                                   b2r_sb[:], OP.mult, OP.add)
                    nc.sync.dma_start(out_d[base:base + bs, :], ofb[:bs, :])

                _edge_layer(nc, tc, meta, 2, (sbm, sbg, sbs, sbw, psb, psa),
                            ad2, hb2, hfull2, writer_d)

    nc.compile()
    return nc


# --------------------------------------------------------------------------
# entry point
# --------------------------------------------------------------------------

def kernel(x, edge_index, W1, a_src1, a_dst1, b1, W2, a_src2, a_dst2, b2,
           _trace=False):
    in_maps, meta, perm_order = _prepare(
        x, edge_index, W1, a_src1, a_dst1, b1, W2, a_src2, a_dst2, b2)

    import time as _time
    _t0 = _time.time()
    key = (meta["TT"], tuple(meta["T_lo"]), tuple(meta["T_hi"]),
           tuple(meta["g_lo"]), tuple(meta["g_hi"]))
    if key not in _cache:
        _cache.clear()
        _cache[key] = _build(meta)
    nc = _cache[key]
    print(f"[kernel] build done at {_time.time()-_t0:.1f}s", flush=True)

    kw = {}
    if _trace:
        kw = dict(trace=True)
    res = bass_utils.run_bass_kernel_spmd(nc, in_maps, core_ids=list(range(C)), **kw)

    out = np.empty((N, HID), np.float32)
    for c in range(C):
        out[perm_order[c * NS:(c + 1) * NS]] = res.results[c]["out"]
    kernel._last_result = res
    return out
